# revision 4
# baseline (speedup 1.0000x reference)
"""MinGRU forward on 8 TRN2 NeuronCores.

Math (linear-space reformulation of the reference's log-space Heinsen scan):
    hg = x @ W_hg.T                       # [B,S,2D]
    hidden, gate = split(hg)
    z = sigmoid(gate)
    c = sigmoid(-gate)                    # = 1 - z = exp(-softplus(gate))
    g = max(hidden + 0.5, sigmoid(hidden))  # == where(h>=0, h+0.5, sigmoid(h)) exactly
    u = z * g
    h[t] = c[t] * h[t-1] + u[t]           # convex combination -> bounded, stable
    out = h

The recurrence maps directly onto the DVE `tensor_tensor_scan` instruction
(state = data0*state + data1 along the free dim, fp32 internal state).

Sharding: 8 cores = 4 batches x 2 feature-halves (512 features each).
No cross-core communication: the scan is per-feature independent.
Host pre-transposes x (-> xT [D,S]) and W (-> wT [D, 2*512]) so the kernel
needs no on-chip transposes; matmul uses fp32r (fp32 with 11-bit mantissa).

v2: the opening (W + first x chunks, ~8 MB) was HBM-bound and the PE clock
ramp (HAM gate, full speed only after ~3us of CONTINUOUS PE activity) was
repeatedly reset by DMA-starve gaps.  Fix: ship W and the first two seq
chunks as bf16 (half the bytes), upconvert to fp32 on otherwise-idle
engines (scalar/gpsimd/vector) as granules land, and start with a 256-wide
chunk so less work happens at ramp clock.  Matmuls stay fp32r (bf16 tiles
are upcast; fp32r needs width>=256 for the 1 cycle/row fast path).
"""

import numpy as np

B, S, D = 4, 4096, 1024
DH = D // 2          # features per core
N_CORES = 8
KC = 128             # contraction chunk
NKC = D // KC        # 8 k chunks
FC = 128             # feature chunk (psum partitions)
NFC = DH // FC       # 4 feature chunks

WIDTHS = [256, 512, 512, 512, 512, 512, 512, 512, 256]
assert sum(WIDTHS) == S
HEAD = 2                              # chunks served from the bf16 x tensor
HEADW = sum(WIDTHS[:HEAD])            # 768

_CACHE = {}

CONFIG = {
    "xbufs": 2,
    "psbufs": 4,
    "ebufs": 2,
    "split_last_scan": True,  # last chunk: 2 chained half-scans so out-DMA overlaps
}


def _build():
    import concourse.bacc as bacc
    import concourse.tile as tile
    import concourse.mybir as mybir

    f32 = mybir.dt.float32
    fmm = mybir.dt.float32r   # matmul operand dtype
    bf16 = mybir.dt.bfloat16
    AF = mybir.ActivationFunctionType
    OP = mybir.AluOpType

    nc = bacc.Bacc("TRN2")
    # bf16 head of x: first HEADW tokens (opening is HBM-bound; halve bytes)
    xTh = nc.dram_tensor("xTh", [D, HEADW], bf16, kind="ExternalInput")
    xT = nc.dram_tensor("xT", [D, S], fmm, kind="ExternalInput")
    # wT layout: [D, NFC, 2*FC] bf16 — per feature-chunk fc, 128 hidden cols
    # then 128 gate cols, contiguous.
    wT = nc.dram_tensor("wT", [D, NFC, 2 * FC], bf16, kind="ExternalInput")
    outT = nc.dram_tensor("outT", [DH, S], f32, kind="ExternalOutput")

    with tile.TileContext(nc) as tc:
        with (
            tc.tile_pool(name="w", bufs=1) as wpool,
            tc.tile_pool(name="wb", bufs=1) as wbpool,
            tc.tile_pool(name="xb", bufs=1) as xbpool,
            tc.tile_pool(name="xh", bufs=1) as xhpool,
            tc.tile_pool(name="x", bufs=CONFIG["xbufs"]) as xpool,
            tc.tile_pool(name="ew", bufs=CONFIG["ebufs"]) as epool,
            tc.tile_pool(name="h", bufs=2) as hpool,
            tc.tile_pool(name="ps", bufs=CONFIG["psbufs"], space="PSUM") as pspool,
        ):
            wts, wbf = [], []
            for fc in range(NFC):
                wtf = wpool.tile([KC, NKC, 2 * FC], fmm, tag=f"w{fc}")
                wts.append(wtf)
                wbtf = wbpool.tile([KC, NKC, 2 * FC], bf16, tag=f"wb{fc}")
                wbf.append(wbtf)

            offs = np.concatenate([[0], np.cumsum(WIDTHS)]).astype(int)

            # ---- opening DMAs -------------------------------------------
            # The critical chain to the first matmul is: preamble barrier
            # (~7.2us, fixed) -> sync-ring first packets (~1.6us after
            # doorbell) -> W fc0 k0-1 (128KB bf16) + x0 k0-1 (128KB bf16)
            # -> upconvert -> matmul.  Those two granules go FIRST on the
            # sync ring; the rest of W streams on the scalar (ACT) ring.
            xT_h = xTh.rearrange("(k p) s -> p k s", p=KC)
            wT_r = [wT[:, fc, :].rearrange("(k p) e -> p k e", p=KC) for fc in range(NFC)]

            # fc0 in k-quarters: q1 on sync (gates first matmul), q2-4 on
            # scalar ring ahead of fc1-3 halves (consumed mid-sc0).
            nc.sync.dma_start(wbf[0][:, 0:2, :], wT_r[0][:, 0:2, :])
            xb0 = xbpool.tile([KC, NKC, WIDTHS[0]], bf16, tag="xb0")
            for k2 in range(NKC // 2):
                nc.sync.dma_start(
                    xb0[:, 2 * k2:2 * k2 + 2, :], xT_h[:, 2 * k2:2 * k2 + 2, 0:WIDTHS[0]]
                )
            xb1 = xbpool.tile([KC, NKC, WIDTHS[1]], bf16, tag="xb1")
            for hlf in range(2):
                nc.sync.dma_start(
                    xb1[:, 4 * hlf:4 * hlf + 4, :],
                    xT_h[:, 4 * hlf:4 * hlf + 4, offs[1]:offs[2]],
                )

            nc.scalar.dma_start(wbf[0][:, 2:4, :], wT_r[0][:, 2:4, :])
            nc.scalar.dma_start(wbf[0][:, 4:6, :], wT_r[0][:, 4:6, :])
            nc.scalar.dma_start(wbf[0][:, 6:8, :], wT_r[0][:, 6:8, :])
            for fc in range(1, NFC):
                nc.scalar.dma_start(wbf[fc][:, 0:4, :], wT_r[fc][:, 0:4, :])
                nc.scalar.dma_start(wbf[fc][:, 4:8, :], wT_r[fc][:, 4:8, :])

            # ---- upconversions ------------------------------------------
            # scalar (ACT) engine: the granule gating the first matmul, then
            # fc1-3 first halves (needed at sc0 fc1/2/3 start).
            nc.scalar.copy(wts[0][:, 0:2, :], wbf[0][:, 0:2, :])
            # gpsimd: fc0 remainder + fc1-3 second halves.
            nc.gpsimd.tensor_scalar_add(wts[0][:, 2:4, :], wbf[0][:, 2:4, :], 0.0)
            nc.gpsimd.tensor_scalar_add(wts[0][:, 4:6, :], wbf[0][:, 4:6, :], 0.0)
            nc.gpsimd.tensor_scalar_add(wts[0][:, 6:8, :], wbf[0][:, 6:8, :], 0.0)
            for fc in range(1, NFC):
                nc.scalar.copy(wts[fc][:, 0:4, :], wbf[fc][:, 0:4, :])
                nc.gpsimd.tensor_scalar_add(wts[fc][:, 4:8, :], wbf[fc][:, 4:8, :], 0.0)
            # vector: x head chunks (k-pair granules for sc0 to chase the
            # first matmuls; halves for sc1).
            xh0 = xhpool.tile([KC, NKC, WIDTHS[0]], fmm, tag="xh0")
            for k2 in range(NKC // 2):
                nc.vector.tensor_scalar_add(
                    xh0[:, 2 * k2:2 * k2 + 2, :], xb0[:, 2 * k2:2 * k2 + 2, :], 0.0
                )
            xh1 = xhpool.tile([KC, NKC, WIDTHS[1]], fmm, tag="xh1")
            for hlf in range(2):
                nc.vector.tensor_scalar_add(
                    xh1[:, 4 * hlf:4 * hlf + 4, :], xb1[:, 4 * hlf:4 * hlf + 4, :], 0.0
                )

            # Serial issue order measured fastest: cross-chunk reorderings
            # that chase earlier W/x deadlines perturb the DMA queues.
            order = [(sc, fc) for sc in range(len(WIDTHS)) for fc in range(NFC)]

            hprev = [None] * NFC
            xts = {0: xh0, 1: xh1}
            for sc, fc in order:
                width = WIDTHS[sc]
                off = int(offs[sc])
                if sc in xts:
                    xt = xts[sc]
                else:
                    xt = xpool.tile([KC, NKC, width], fmm, tag="xt")
                    xT_r = xT[:, off:off + width].rearrange("(k p) s -> p k s", p=KC)
                    nc.sync.dma_start(xt[:], xT_r)
                    xts[sc] = xt
                ph = pspool.tile([FC, width], f32, tag="ph")
                pg = pspool.tile([FC, width], f32, tag="pg")
                if sc == 0:
                    # interleave h/g per k-slice: each newly-landed slice
                    # feeds 2 matmuls, halving PE stalls while DMA-gated
                    for k in range(NKC):
                        nc.tensor.matmul(
                            ph[:], wts[fc][:, k, 0:FC], xt[:, k, :],
                            start=(k == 0), stop=(k == NKC - 1),
                        )
                        nc.tensor.matmul(
                            pg[:], wts[fc][:, k, FC:2 * FC], xt[:, k, :],
                            start=(k == 0), stop=(k == NKC - 1),
                        )
                else:
                    for k in range(NKC):
                        nc.tensor.matmul(
                            ph[:], wts[fc][:, k, 0:FC], xt[:, k, :],
                            start=(k == 0), stop=(k == NKC - 1),
                        )
                    for k in range(NKC):
                        nc.tensor.matmul(
                            pg[:], wts[fc][:, k, FC:2 * FC], xt[:, k, :],
                            start=(k == 0), stop=(k == NKC - 1),
                        )
                zt = epool.tile([FC, width], f32, tag="z")
                ct = epool.tile([FC, width], f32, tag="c")
                st = epool.tile([FC, width], f32, tag="s")
                gt = epool.tile([FC, width], f32, tag="g")
                ut = epool.tile([FC, width], f32, tag="u")
                # s first: it heads the DVE critical chain (s->g->u->scan)
                nc.scalar.activation(st[:], ph[:], AF.Sigmoid)
                nc.scalar.activation(zt[:], pg[:], AF.Sigmoid)
                nc.scalar.activation(ct[:], pg[:], AF.Sigmoid, scale=-1.0)
                # g = (hidden + 0.5) max sigmoid(hidden)
                nc.vector.scalar_tensor_tensor(
                    gt[:], ph[:], 0.5, st[:], op0=OP.add, op1=OP.max
                )
                nc.vector.tensor_mul(ut[:], zt[:], gt[:])
                ht = hpool.tile([FC, width], f32, tag=f"h{fc}")
                pw = WIDTHS[sc - 1]
                init = 0.0 if sc == 0 else hprev[fc][:, pw - 1:pw]
                if CONFIG["split_last_scan"] and sc == len(WIDTHS) - 1:
                    hw_ = width // 2
                    nc.vector.tensor_tensor_scan(
                        ht[:, 0:hw_], ct[:, 0:hw_], ut[:, 0:hw_], init,
                        op0=OP.mult, op1=OP.add,
                    )
                    nc.sync.dma_start(
                        outT[fc * FC:(fc + 1) * FC, off:off + hw_], ht[:, 0:hw_]
                    )
                    nc.vector.tensor_tensor_scan(
                        ht[:, hw_:width], ct[:, hw_:width], ut[:, hw_:width],
                        ht[:, hw_ - 1:hw_], op0=OP.mult, op1=OP.add,
                    )
                    nc.sync.dma_start(
                        outT[fc * FC:(fc + 1) * FC, off + hw_:off + width],
                        ht[:, hw_:width],
                    )
                    hprev[fc] = ht
                else:
                    nc.vector.tensor_tensor_scan(
                        ht[:], ct[:], ut[:], init, op0=OP.mult, op1=OP.add
                    )
                    hprev[fc] = ht
                    nc.sync.dma_start(
                        outT[fc * FC:(fc + 1) * FC, off:off + width], ht[:]
                    )

    nc.compile()
    return nc


def _round_fp32r(a: np.ndarray) -> np.ndarray:
    """Round fp32 array to fp32r (11 explicit mantissa bits) with RNE."""
    u = np.ascontiguousarray(a, dtype=np.float32).view(np.uint32)
    r = (u + np.uint32(0x7FF) + ((u >> np.uint32(12)) & np.uint32(1))) & np.uint32(0xFFFFF000)
    return r.view(np.float32)


def _to_bf16(a: np.ndarray):
    import ml_dtypes
    return np.ascontiguousarray(a, dtype=np.float32).astype(ml_dtypes.bfloat16)


def _prep_in_maps(x: np.ndarray, W_hg: np.ndarray):
    x = np.asarray(x, dtype=np.float32)
    W_hg = np.asarray(W_hg, dtype=np.float32)
    xTs = [np.ascontiguousarray(x[b].T) for b in range(B)]
    xTfs = [_round_fp32r(t) for t in xTs]
    xThs = [_to_bf16(t[:, :HEADW]) for t in xTs]
    wTs = []
    for c in range(2):
        # [D, NFC, 2*FC]: per fc, 128 hidden cols then 128 gate cols
        wt = np.empty((D, NFC, 2 * FC), dtype=np.float32)
        for fc in range(NFC):
            rows_h = W_hg[c * DH + fc * FC:c * DH + (fc + 1) * FC]      # [FC, D]
            rows_g = W_hg[D + c * DH + fc * FC:D + c * DH + (fc + 1) * FC]
            wt[:, fc, 0:FC] = rows_h.T
            wt[:, fc, FC:2 * FC] = rows_g.T
        wTs.append(_to_bf16(wt))
    return [
        {"xTh": xThs[core // 2], "xT": xTfs[core // 2], "wT": wTs[core % 2]}
        for core in range(N_CORES)
    ]


def _get_runner():
    """Build the Bass module once and cache a compiled jax callable for it.

    Mirrors bass2jax.run_bass_via_pjrt's multi-core path, but keeps the
    jitted/sharded executable so repeat kernel() calls skip re-tracing.
    """
    if "runner" in _CACHE:
        return _CACHE["runner"]

    import jax
    from jax.experimental.shard_map import shard_map
    from jax.sharding import Mesh, PartitionSpec
    from concourse import bass2jax

    if "nc" not in _CACHE:
        _CACHE["nc"] = _build()
    nc = _CACHE["nc"]
    bass2jax.install_neuronx_cc_hook()

    in_names = ["xTh", "xT", "wT"]
    out_name = "outT"
    out_shape, out_dtype = (DH, S), np.float32
    partition_name = nc.partition_id_tensor.name if nc.partition_id_tensor else None

    def _body(xTh, xT, wT, zout):
        operands = [xTh, xT, wT, zout]
        if partition_name is not None:
            operands.append(bass2jax.partition_id_tensor())
        outs = bass2jax._bass_exec_p.bind(
            *operands,
            out_avals=(jax.core.ShapedArray(out_shape, out_dtype),),
            in_names=tuple(in_names + [out_name] + ([partition_name] if partition_name else [])),
            out_names=(out_name,),
            lowering_input_output_aliases=(),
            sim_require_finite=True,
            sim_require_nnan=True,
            nc=nc,
        )
        return tuple(outs)

    devices = jax.devices()[:N_CORES]
    mesh = Mesh(np.asarray(devices), ("core",))
    sharded = jax.jit(
        shard_map(
            _body, mesh=mesh,
            in_specs=(PartitionSpec("core"),) * 4,
            out_specs=(PartitionSpec("core"),),
            check_rep=False,
        ),
        donate_argnums=(3,),
        keep_unused=True,
    )

    def run(in_maps):
        concat_xh = np.concatenate([m["xTh"] for m in in_maps], axis=0)
        concat_x = np.concatenate([m["xT"] for m in in_maps], axis=0)
        concat_w = np.concatenate([m["wT"] for m in in_maps], axis=0)
        zeros = np.zeros((N_CORES * DH, S), np.float32)
        (out_arr,) = sharded(concat_xh, concat_x, concat_w, zeros)
        return np.asarray(out_arr).reshape(N_CORES, DH, S)

    _CACHE["runner"] = run
    return run


def kernel(x: np.ndarray, W_hg: np.ndarray) -> np.ndarray:
    run = _get_runner()
    in_maps = _prep_in_maps(x, W_hg)
    outs = run(in_maps)

    out = np.empty((B, S, D), dtype=np.float32)
    for core in range(N_CORES):
        b, c = core // 2, core % 2
        out[b, :, c * DH:(c + 1) * DH] = outs[core].T
    return out


# revision 6
# speedup vs baseline: 1.3536x; 1.3536x over previous
"""MinGRU forward on 8 TRN2 NeuronCores.

Math (linear-space reformulation of the reference's log-space Heinsen scan):
    hg = x @ W_hg.T                       # [B,S,2D]
    hidden, gate = split(hg)
    z = sigmoid(gate)
    c = sigmoid(-gate)                    # = 1 - z = exp(-softplus(gate))
    g = max(hidden + 0.5, sigmoid(hidden))  # == where(h>=0, h+0.5, sigmoid(h)) exactly
    u = z * g
    h[t] = c[t] * h[t-1] + u[t]           # convex combination -> bounded, stable
    out = h

The recurrence maps directly onto the DVE `tensor_tensor_scan` instruction
(state = data0*state + data1 along the free dim, fp32 internal state).

Sharding: 8 cores = 4 batches x 2 feature-halves (512 features each).
No cross-core communication: the scan is per-feature independent.
Host pre-transposes x (-> xT [D,S]) and W (-> wT [D, 2*512]) so the kernel
needs no on-chip transposes; matmul uses fp32r (fp32 with 11-bit mantissa).
Measured on HW: fp32r 512-col matmuls net 227 ns vs fp16's 259 ns — fp16
streams SLOWER despite half the bytes, so fp32r it is.  Inputs are
pre-rounded to fp32r on the host (RNE).

v3: pair-blocked unit order.  The opening is HBM-bound (W 4.2MB + x0 + x1
before the PE can stream without gaps at ~358 GB/s/core), and every PE
stall resets the HAM clock-ramp timer (full clock only after ~3us of
CONTINUOUS PE activity).  Processing units in (sc-pair x fc-pair) blocks
means the first 4 units only need W fc0+fc1 (2.1MB) + x0 + x1, doubling
every W/x DMA deadline; after the first matmul the PE never starves.
(bf16 shipping was tried and abandoned: DVE/GpSimd writes with fp32r
output dtype run ~18x slow, and bf16 W costs 10x accuracy.)
"""

import numpy as np

B, S, D = 4, 4096, 1024
DH = D // 2          # features per core
N_CORES = 8
SC = 512             # tokens per seq chunk (PSUM bank = 512 fp32)
KC = 128             # contraction chunk
NKC = D // KC        # 8 k chunks
FC = 128             # feature chunk (psum partitions)
NFC = DH // FC       # 4 feature chunks

_CACHE = {}

CONFIG = {
    "xbufs": 4,            # pair-blocking: x tiles live across 2 fc-blocks
    "psbufs": 4,
    "ebufs": 2,
    "split_last_scan": True,  # last chunk: 2 chained half-scans so out-DMA overlaps
}

WIDTHS = [512, 512, 512, 512, 512, 512, 512, 256, 256]
assert sum(WIDTHS) == S


def _unit_order(n_sc):
    """(sc, fc) units in (sc-pair x fc-pair) blocks.

    Block = 4 units sharing 2 W tiles + 2 x chunks; halves the W working
    set the opening must deliver before the PE can run uninterrupted.
    """
    order = []
    scps = [(a, a + 1) if a + 1 < n_sc else (a,) for a in range(0, n_sc, 2)]
    for scp in scps:
        for fcp in ((0, 1), (2, 3)):
            for sc in scp:
                for fc in fcp:
                    order.append((sc, fc))
    return order


def _build():
    import concourse.bacc as bacc
    import concourse.tile as tile
    import concourse.mybir as mybir

    f32 = mybir.dt.float32
    fmm = mybir.dt.float32r   # matmul operand dtype
    AF = mybir.ActivationFunctionType
    OP = mybir.AluOpType

    nc = bacc.Bacc("TRN2")
    xT = nc.dram_tensor("xT", [D, S], fmm, kind="ExternalInput")
    # wT layout: [D, NFC, 2*FC] — per feature-chunk fc, 128 hidden cols then
    # 128 gate cols, contiguous, so each fc's weights are one 1 MiB DMA.
    wT = nc.dram_tensor("wT", [D, NFC, 2 * FC], fmm, kind="ExternalInput")
    outT = nc.dram_tensor("outT", [DH, S], f32, kind="ExternalOutput")

    with tile.TileContext(nc) as tc:
        with (
            tc.tile_pool(name="w", bufs=1) as wpool,
            tc.tile_pool(name="x", bufs=CONFIG["xbufs"]) as xpool,
            tc.tile_pool(name="ew", bufs=CONFIG["ebufs"]) as epool,
            tc.tile_pool(name="h", bufs=2) as hpool,
            tc.tile_pool(name="ps", bufs=CONFIG["psbufs"], space="PSUM") as pspool,
        ):
            wts = []
            for fc in range(NFC):
                wtf = wpool.tile([KC, NKC, 2 * FC], fmm, tag=f"w{fc}")
                wts.append(wtf)

            offs = np.concatenate([[0], np.cumsum(WIDTHS)]).astype(int)
            wT_r = [wT[:, fc, :].rearrange("(k p) e -> p k e", p=KC) for fc in range(NFC)]

            # Opening DMAs.  Critical chain to the first matmul: preamble
            # barrier (~7.2us, fixed) -> sync ring first packet (~1.6us
            # after doorbell) -> W fc0 k0-1 (256KB) + x0 k0-1 (512KB).
            # Interleave W fc0 quarters with x0 k-pairs on the sync ring so
            # the k-interleaved sc0 matmuls chase the granules; everything
            # else streams on the scalar (ACT) ring, whose deadlines the
            # pair-blocked order doubles.
            xt0 = xpool.tile([KC, NKC, WIDTHS[0]], fmm, tag="xt")
            xT_r0 = xT[:, 0:WIDTHS[0]].rearrange("(k p) s -> p k s", p=KC)
            nc.sync.dma_start(wts[0][:, 0:2, :], wT_r[0][:, 0:2, :])
            nc.sync.dma_start(xt0[:, 0:1, :], xT_r0[:, 0:1, :])
            nc.sync.dma_start(xt0[:, 1:2, :], xT_r0[:, 1:2, :])
            nc.sync.dma_start(wts[0][:, 2:4, :], wT_r[0][:, 2:4, :])
            nc.sync.dma_start(xt0[:, 2:4, :], xT_r0[:, 2:4, :])
            nc.sync.dma_start(xt0[:, 4:8, :], xT_r0[:, 4:8, :])
            xt1 = xpool.tile([KC, NKC, WIDTHS[1]], fmm, tag="xt")
            xT_r1 = xT[:, offs[1]:offs[2]].rearrange("(k p) s -> p k s", p=KC)
            nc.sync.dma_start(xt1[:, 0:4, :], xT_r1[:, 0:4, :])
            nc.sync.dma_start(xt1[:, 4:8, :], xT_r1[:, 4:8, :])

            nc.scalar.dma_start(wts[0][:, 4:8, :], wT_r[0][:, 4:8, :])
            for fc in range(1, NFC):
                nc.scalar.dma_start(wts[fc][:, 0:4, :], wT_r[fc][:, 0:4, :])
                nc.scalar.dma_start(wts[fc][:, 4:8, :], wT_r[fc][:, 4:8, :])

            order = _unit_order(len(WIDTHS))

            hprev = [None] * NFC
            xts = {0: xt0, 1: xt1}
            for sc, fc in order:
                width = WIDTHS[sc]
                off = int(offs[sc])
                if sc in xts:
                    xt = xts[sc]
                else:
                    xt = xpool.tile([KC, NKC, width], fmm, tag="xt")
                    xT_r = xT[:, off:off + width].rearrange("(k p) s -> p k s", p=KC)
                    nc.sync.dma_start(xt[:], xT_r)
                    xts[sc] = xt
                ph = pspool.tile([FC, width], f32, tag="ph")
                pg = pspool.tile([FC, width], f32, tag="pg")
                if sc == 0:
                    # interleave h/g per k-slice: each newly-landed slice
                    # feeds 2 matmuls, halving PE stalls while DMA-gated
                    for k in range(NKC):
                        nc.tensor.matmul(
                            ph[:], wts[fc][:, k, 0:FC], xt[:, k, :],
                            start=(k == 0), stop=(k == NKC - 1),
                        )
                        nc.tensor.matmul(
                            pg[:], wts[fc][:, k, FC:2 * FC], xt[:, k, :],
                            start=(k == 0), stop=(k == NKC - 1),
                        )
                else:
                    for k in range(NKC):
                        nc.tensor.matmul(
                            ph[:], wts[fc][:, k, 0:FC], xt[:, k, :],
                            start=(k == 0), stop=(k == NKC - 1),
                        )
                    for k in range(NKC):
                        nc.tensor.matmul(
                            pg[:], wts[fc][:, k, FC:2 * FC], xt[:, k, :],
                            start=(k == 0), stop=(k == NKC - 1),
                        )
                zt = epool.tile([FC, width], f32, tag="z")
                ct = epool.tile([FC, width], f32, tag="c")
                st = epool.tile([FC, width], f32, tag="s")
                gt = epool.tile([FC, width], f32, tag="g")
                ut = epool.tile([FC, width], f32, tag="u")
                # s first: it heads the DVE critical chain (s->g->u->scan)
                nc.scalar.activation(st[:], ph[:], AF.Sigmoid)
                nc.scalar.activation(zt[:], pg[:], AF.Sigmoid)
                nc.scalar.activation(ct[:], pg[:], AF.Sigmoid, scale=-1.0)
                # g = (hidden + 0.5) max sigmoid(hidden)
                nc.vector.scalar_tensor_tensor(
                    gt[:], ph[:], 0.5, st[:], op0=OP.add, op1=OP.max
                )
                nc.vector.tensor_mul(ut[:], zt[:], gt[:])
                ht = hpool.tile([FC, width], f32, tag=f"h{fc}")
                pw = WIDTHS[sc - 1]
                init = 0.0 if sc == 0 else hprev[fc][:, pw - 1:pw]
                if CONFIG["split_last_scan"] and sc == len(WIDTHS) - 1:
                    hw_ = width // 2
                    nc.vector.tensor_tensor_scan(
                        ht[:, 0:hw_], ct[:, 0:hw_], ut[:, 0:hw_], init,
                        op0=OP.mult, op1=OP.add,
                    )
                    nc.sync.dma_start(
                        outT[fc * FC:(fc + 1) * FC, off:off + hw_], ht[:, 0:hw_]
                    )
                    nc.vector.tensor_tensor_scan(
                        ht[:, hw_:width], ct[:, hw_:width], ut[:, hw_:width],
                        ht[:, hw_ - 1:hw_], op0=OP.mult, op1=OP.add,
                    )
                    nc.sync.dma_start(
                        outT[fc * FC:(fc + 1) * FC, off + hw_:off + width],
                        ht[:, hw_:width],
                    )
                    hprev[fc] = ht
                else:
                    nc.vector.tensor_tensor_scan(
                        ht[:], ct[:], ut[:], init, op0=OP.mult, op1=OP.add
                    )
                    hprev[fc] = ht
                    nc.sync.dma_start(
                        outT[fc * FC:(fc + 1) * FC, off:off + width], ht[:]
                    )

    nc.compile()
    return nc


def _round_fp32r(a: np.ndarray) -> np.ndarray:
    """Round fp32 array to fp32r (11 explicit mantissa bits) with RNE."""
    u = np.ascontiguousarray(a, dtype=np.float32).view(np.uint32)
    r = (u + np.uint32(0x7FF) + ((u >> np.uint32(12)) & np.uint32(1))) & np.uint32(0xFFFFF000)
    return r.view(np.float32)


def _prep_in_maps(x: np.ndarray, W_hg: np.ndarray):
    x = np.asarray(x, dtype=np.float32)
    W_hg = np.asarray(W_hg, dtype=np.float32)
    xTs = [_round_fp32r(np.ascontiguousarray(x[b].T)) for b in range(B)]
    wTs = []
    for c in range(2):
        # [D, NFC, 2*FC]: per fc, 128 hidden cols then 128 gate cols
        wt = np.empty((D, NFC, 2 * FC), dtype=np.float32)
        for fc in range(NFC):
            rows_h = W_hg[c * DH + fc * FC:c * DH + (fc + 1) * FC]      # [FC, D]
            rows_g = W_hg[D + c * DH + fc * FC:D + c * DH + (fc + 1) * FC]
            wt[:, fc, 0:FC] = rows_h.T
            wt[:, fc, FC:2 * FC] = rows_g.T
        wTs.append(_round_fp32r(wt))
    return [{"xT": xTs[core // 2], "wT": wTs[core % 2]} for core in range(N_CORES)]


def _get_runner():
    """Build the Bass module once and cache a compiled jax callable for it.

    Mirrors bass2jax.run_bass_via_pjrt's multi-core path, but keeps the
    jitted/sharded executable so repeat kernel() calls skip re-tracing.
    """
    if "runner" in _CACHE:
        return _CACHE["runner"]

    import jax
    from jax.experimental.shard_map import shard_map
    from jax.sharding import Mesh, PartitionSpec
    from concourse import bass2jax

    if "nc" not in _CACHE:
        _CACHE["nc"] = _build()
    nc = _CACHE["nc"]
    bass2jax.install_neuronx_cc_hook()

    in_names = ["xT", "wT"]
    out_name = "outT"
    out_shape, out_dtype = (DH, S), np.float32
    partition_name = nc.partition_id_tensor.name if nc.partition_id_tensor else None

    def _body(xT, wT, zout):
        operands = [xT, wT, zout]
        if partition_name is not None:
            operands.append(bass2jax.partition_id_tensor())
        outs = bass2jax._bass_exec_p.bind(
            *operands,
            out_avals=(jax.core.ShapedArray(out_shape, out_dtype),),
            in_names=tuple(in_names + [out_name] + ([partition_name] if partition_name else [])),
            out_names=(out_name,),
            lowering_input_output_aliases=(),
            sim_require_finite=True,
            sim_require_nnan=True,
            nc=nc,
        )
        return tuple(outs)

    devices = jax.devices()[:N_CORES]
    mesh = Mesh(np.asarray(devices), ("core",))
    sharded = jax.jit(
        shard_map(
            _body, mesh=mesh,
            in_specs=(PartitionSpec("core"),) * 3,
            out_specs=(PartitionSpec("core"),),
            check_rep=False,
        ),
        donate_argnums=(2,),
        keep_unused=True,
    )

    def run(in_maps):
        concat_x = np.concatenate([m["xT"] for m in in_maps], axis=0)
        concat_w = np.concatenate([m["wT"] for m in in_maps], axis=0)
        zeros = np.zeros((N_CORES * DH, S), np.float32)
        (out_arr,) = sharded(concat_x, concat_w, zeros)
        return np.asarray(out_arr).reshape(N_CORES, DH, S)

    _CACHE["runner"] = run
    return run


def kernel(x: np.ndarray, W_hg: np.ndarray) -> np.ndarray:
    run = _get_runner()
    in_maps = _prep_in_maps(x, W_hg)
    outs = run(in_maps)

    out = np.empty((B, S, D), dtype=np.float32)
    for core in range(N_CORES):
        b, c = core // 2, core % 2
        out[b, :, c * DH:(c + 1) * DH] = outs[core].T
    return out
